# revision 1
# baseline (speedup 1.0000x reference)
"""DGCNN kernel for Trainium2 (Bass/Tile), data-parallel over batch across 8 cores.

Key algorithmic decomposition (per core, one point cloud of N=2048 points):
  EdgeConv(max_k relu(bn(W @ [nb - ctr; ctr]))) decomposes as
    h[n,k,o] = Wn·p_{idx(n,k)} + (Wc - Wn)·p_n         (Wn = W[:, :C], Wc = W[:, C:])
  and since BN scale > 0 and relu is monotonic,
    out[n] = relu(bn(max_k (Wn·p_idx) + Wd·p_n))       (Wd = Wc - Wn)
  So per layer: Y = P @ WnT (one matmul over points), gather+max over the 20
  kNN rows of Y, add the center term, bn+relu.

  kNN: keys[n,m] = 2*(p_n·p_m) - |p_n|^2 - |p_m|^2 (= -d2), built fully on the
  PE via rank-1 correction matmuls; exact top-24 per row via 3 rounds of DVE
  max8 / max_index / match_replace; top-20 = first 20 (sorted desc).

  Gather runs on GPSIMD (ap_gather) against Y^T [Cout, N]; the index list is
  rewrapped to the per-core [16, S] partition-interleaved layout with two
  small SBUF->SBUF DMA steps (partition fold + replicate).
"""

import sys

import numpy as np

sys.path.insert(0, "/opt/trn_rl_repo")

EPS = 1e-5
BN_SCALE = np.float32(1.0 / np.sqrt(1.0 + EPS))
N = 2048
K = 20
KR = 24  # extracted per row (3 rounds of max8)
NCORES = 8
DIMS = [(3, 64), (64, 64), (64, 128), (128, 256)]  # (Cin, Cout) per edge conv

_CACHE = {}


def _build_module():
    from concourse import bacc, mybir, tile

    dt = mybir.dt
    f32 = dt.float32
    u16 = dt.uint16
    i16 = dt.int16
    AF = mybir.ActivationFunctionType
    ALU = mybir.AluOpType
    AX = mybir.AxisListType

    nc = bacc.Bacc("TRN2", target_bir_lowering=False, debug=False)

    # ---------------- DRAM I/O ----------------
    X = nc.dram_tensor("xq", [3, N], f32, kind="ExternalInput")
    conv_w = []
    for li, (ci, co) in enumerate(DIMS):
        nb = (co + 127) // 128
        cb = min(co, 128)
        conv_w.append(
            dict(
                wnT=nc.dram_tensor(f"wn{li}", [ci, co], f32, kind="ExternalInput"),
                wdT=nc.dram_tensor(f"wd{li}", [ci, co], f32, kind="ExternalInput"),
                gs=nc.dram_tensor(f"gs{li}", [cb, nb], f32, kind="ExternalInput"),
                bb=nc.dram_tensor(f"bb{li}", [cb, nb], f32, kind="ExternalInput"),
            )
        )
    W5 = nc.dram_tensor("w5t", [128, 5, 1024], f32, kind="ExternalInput")
    G5 = nc.dram_tensor("g5s", [128, 8], f32, kind="ExternalInput")
    B5 = nc.dram_tensor("b5s", [128, 8], f32, kind="ExternalInput")
    WL1 = nc.dram_tensor("wl1t", [128, 16, 512], f32, kind="ExternalInput")
    G6 = nc.dram_tensor("g6s", [128, 4], f32, kind="ExternalInput")
    B6 = nc.dram_tensor("b6s", [128, 4], f32, kind="ExternalInput")
    WL2 = nc.dram_tensor("wl2t", [128, 4, 256], f32, kind="ExternalInput")
    G7 = nc.dram_tensor("g7s", [128, 2], f32, kind="ExternalInput")
    BI2 = nc.dram_tensor("bi2", [128, 2], f32, kind="ExternalInput")
    WL3 = nc.dram_tensor("wl3t", [128, 2, 40], f32, kind="ExternalInput")
    BL3 = nc.dram_tensor("bl3s", [40, 1], f32, kind="ExternalInput")
    OUT = nc.dram_tensor("outq", [40, 1], f32, kind="ExternalOutput")

    with tile.TileContext(nc) as tc:
        with (
            tc.tile_pool(name="const", bufs=1) as constp,
            tc.tile_pool(name="wts", bufs=1) as wts,
            tc.tile_pool(name="feat", bufs=1) as featp,
            tc.tile_pool(name="keysp", bufs=2) as keysp,
            tc.tile_pool(name="work", bufs=2) as work,
            tc.tile_pool(name="work1", bufs=1) as work1,
            tc.tile_pool(name="gat", bufs=2) as gatp,
            tc.tile_pool(name="ps", bufs=2, space="PSUM") as ps,
        ):
            # ---------------- constants / weights to SBUF ----------------
            ones1 = constp.tile([1, 512], f32)
            nc.vector.memset(ones1[:], 1.0)
            ones_col = constp.tile([128, 1], f32)
            nc.vector.memset(ones_col[:], 1.0)

            wn_sb, wd_sb, gs_sb, bb_sb = [], [], [], []
            for li, (ci, co) in enumerate(DIMS):
                nb = (co + 127) // 128
                cbp = min(co, 128)
                t_wn = wts.tile([ci, co], f32, tag=f"wn{li}")
                t_wd = wts.tile([ci, co], f32, tag=f"wd{li}")
                t_gs = wts.tile([cbp, nb], f32, tag=f"gs{li}")
                t_bb = wts.tile([cbp, nb], f32, tag=f"bb{li}")
                nc.sync.dma_start(t_wn[:], conv_w[li]["wnT"][:])
                nc.sync.dma_start(t_wd[:], conv_w[li]["wdT"][:])
                nc.sync.dma_start(t_gs[:], conv_w[li]["gs"][:])
                nc.sync.dma_start(t_bb[:], conv_w[li]["bb"][:])
                wn_sb.append(t_wn)
                wd_sb.append(t_wd)
                gs_sb.append(t_gs)
                bb_sb.append(t_bb)

            w5_sb = wts.tile([128, 5, 1024], f32, tag="w5")
            nc.sync.dma_start(w5_sb[:], W5[:])
            g5_sb = wts.tile([128, 8], f32, tag="g5")
            b5_sb = wts.tile([128, 8], f32, tag="b5")
            nc.sync.dma_start(g5_sb[:], G5[:])
            nc.sync.dma_start(b5_sb[:], B5[:])
            wl1_sb = wts.tile([128, 16, 512], f32, tag="wl1")
            nc.sync.dma_start(wl1_sb[:], WL1[:])
            g6_sb = wts.tile([128, 4], f32, tag="g6")
            b6_sb = wts.tile([128, 4], f32, tag="b6")
            nc.sync.dma_start(g6_sb[:], G6[:])
            nc.sync.dma_start(b6_sb[:], B6[:])
            wl2_sb = wts.tile([128, 4, 256], f32, tag="wl2")
            nc.sync.dma_start(wl2_sb[:], WL2[:])
            g7_sb = wts.tile([128, 2], f32, tag="g7")
            bi2_sb = wts.tile([128, 2], f32, tag="bi2")
            nc.sync.dma_start(g7_sb[:], G7[:])
            nc.sync.dma_start(bi2_sb[:], BI2[:])
            wl3_sb = wts.tile([128, 2, 40], f32, tag="wl3")
            nc.sync.dma_start(wl3_sb[:], WL3[:])
            bl3_sb = wts.tile([40, 1], f32, tag="bl3")
            nc.sync.dma_start(bl3_sb[:], BL3[:])

            # input points (layer-0 features), already [C, N]
            pt0 = featp.tile([3, N], f32, tag="ptf3")
            nc.sync.dma_start(pt0[:], X[:])

            # feature tensors for the concat
            f1 = featp.tile([64, N], f32, tag="f1")
            f2 = featp.tile([64, N], f32, tag="f2")
            f3 = featp.tile([128, N], f32, tag="ptf3")
            f4a = featp.tile([128, N], f32, tag="f4a")
            f4b = featp.tile([128, N], f32, tag="f4b")

            def edge_layer(li, PT, out_aps):
                """PT: AP [Cin, N]. out_aps: list of APs [cb, N] per 128-chan block."""
                ci, co = DIMS[li]
                nblk = (co + 127) // 128

                # ---- squared norms row: nsq = -0.5*|p_m|^2 ----
                p2 = work.tile([ci, N], f32, tag="p2")
                nc.scalar.activation(p2[:], PT, AF.Square)
                psq = ps.tile([1, N], f32, tag="ps")
                for j in range(4):
                    sl = slice(j * 512, (j + 1) * 512)
                    nc.tensor.matmul(
                        psq[:, sl], ones_col[0:ci, :], p2[:, sl], start=True, stop=True
                    )
                nsq = work1.tile([1, N], f32, tag="nsq")
                nc.scalar.activation(nsq[:], psq[:], AF.Copy, scale=-0.5)

                # ---- distance keys + exact top-24 per row ----
                dec = work1.tile([128, 16, KR], u16, tag="dec")
                for t in range(16):
                    pg = ps.tile([128, N], f32, tag="ps")
                    tl = slice(t * 128, (t + 1) * 128)
                    for j in range(4):
                        sl = slice(j * 512, (j + 1) * 512)
                        nc.tensor.matmul(
                            pg[:, sl], PT[:, tl], PT[:, sl], start=True, stop=False
                        )
                        nc.tensor.matmul(
                            pg[:, sl], ones1[:, 0:128], nsq[:, sl], start=False, stop=False
                        )
                        nc.tensor.matmul(
                            pg[:, sl], nsq[:, tl], ones1[:, 0:512], start=False, stop=True
                        )
                    keys = keysp.tile([128, N], f32, tag="keys")
                    nc.scalar.activation(keys[:], pg[:], AF.Copy, scale=2.0)
                    for r in range(3):
                        v8 = work.tile([128, 8], f32, tag="v8")
                        nc.vector.max(v8[:], keys[:])
                        nc.vector.max_index(dec[:, t, r * 8 : (r + 1) * 8], v8[:], keys[:])
                        if r < 2:
                            nc.vector.match_replace(keys[:], v8[:], keys[:], -3.0e38)

                # ---- rewrap indices for ap_gather ----
                # w0[p, g, t, k] = dec[16 g + p, t, k]  (partition fold)
                w0 = work1.tile([16, 8, 16, K], i16, tag="w0")
                for g in range(8):
                    nc.sync.dma_start(
                        w0[:, g, :, :], dec[16 * g : 16 * (g + 1), :, 0:K].bitcast(i16)
                    )
                wrep = work1.tile([128, 8, 16, K], i16, tag="wrep")
                for h in range(8):
                    nc.sync.dma_start(wrep[16 * h : 16 * (h + 1), :, :, :], w0[:])

                # ---- per channel-block: Y/c matmuls, gather+max, bn+relu ----
                for blk in range(nblk):
                    cb = min(co - blk * 128, 128)
                    csl = slice(blk * 128, blk * 128 + cb)
                    yt = work1.tile([cb, N], f32, tag="yt")
                    ct = work1.tile([cb, N], f32, tag="ct")
                    pym = ps.tile([cb, N], f32, tag="ps")
                    for j in range(4):
                        sl = slice(j * 512, (j + 1) * 512)
                        nc.tensor.matmul(
                            pym[:, sl], wn_sb[li][:, csl], PT[:, sl], start=True, stop=True
                        )
                    nc.scalar.activation(yt[:], pym[:], AF.Copy)
                    pcm = ps.tile([cb, N], f32, tag="ps")
                    for j in range(4):
                        sl = slice(j * 512, (j + 1) * 512)
                        nc.tensor.matmul(
                            pcm[:, sl], wd_sb[li][:, csl], PT[:, sl], start=True, stop=True
                        )
                    nc.scalar.activation(ct[:], pcm[:], AF.Copy)

                    mt = work1.tile([cb, N], f32, tag="mt")
                    # 16 gather+reduce chunks: (g, half-of-t)
                    mt4 = mt[:].rearrange("c (t g p) -> c t g p", t=16, g=8, p=16)
                    for g in range(8):
                        for th in range(2):
                            gt = gatp.tile([cb, 8, K, 16], f32, tag="gath")
                            idxs = wrep[0:cb, g, th * 8 : (th + 1) * 8, :]
                            nc.gpsimd.ap_gather(
                                gt[:],
                                yt[:],
                                idxs,
                                channels=cb,
                                num_elems=N,
                                d=1,
                                num_idxs=8 * K * 16,
                            )
                            nc.vector.tensor_reduce(
                                mt4[:, th * 8 : (th + 1) * 8, g, :],
                                gt[:].transpose([0, 1, 3, 2]),
                                axis=AX.X,
                                op=ALU.max,
                            )
                    # center term + bn + relu
                    nc.vector.tensor_tensor(mt[:], mt[:], ct[:], ALU.add)
                    nc.scalar.activation(
                        out_aps[blk],
                        mt[:],
                        AF.Relu,
                        bias=bb_sb[li][0:cb, blk : blk + 1],
                        scale=gs_sb[li][0:cb, blk : blk + 1],
                    )

            edge_layer(0, pt0[:], [f1[:]])
            edge_layer(1, f1[:], [f2[:]])
            edge_layer(2, f2[:], [f3[:]])
            edge_layer(3, f3[:], [f4a[:], f4b[:]])

            # ---------------- conv5 (1024) + global max/mean pool ----------------
            pooled = work1.tile([128, 16], f32, tag="pooled")
            rhs_chunks = [f1[:], f2[:], f3[:], f4a[:], f4b[:]]
            chunk_rows = [64, 64, 128, 128, 128]
            for blk in range(8):
                bsl = slice(blk * 128, (blk + 1) * 128)
                ph = ps.tile([128, N], f32, tag="ps")
                for j in range(4):
                    sl = slice(j * 512, (j + 1) * 512)
                    for c in range(5):
                        nc.tensor.matmul(
                            ph[:, sl],
                            w5_sb[0 : chunk_rows[c], c, bsl],
                            rhs_chunks[c][:, sl],
                            start=(c == 0),
                            stop=(c == 4),
                        )
                hb = work.tile([128, N], f32, tag="p2")  # share big-scratch slots
                nc.scalar.activation(
                    hb[:],
                    ph[:],
                    AF.Relu,
                    bias=b5_sb[:, blk : blk + 1],
                    scale=g5_sb[:, blk : blk + 1],
                    accum_out=pooled[:, 8 + blk : 9 + blk],
                )
                nc.vector.tensor_reduce(
                    pooled[:, blk : blk + 1], hb[:], axis=AX.X, op=ALU.max
                )

            # ---------------- MLP head ----------------
            ps1 = ps.tile([128, 4], f32, tag="ps")
            for mb in range(4):
                for c in range(16):
                    nc.tensor.matmul(
                        ps1[:, mb : mb + 1],
                        wl1_sb[:, c, mb * 128 : (mb + 1) * 128],
                        pooled[:, c : c + 1],
                        start=(c == 0),
                        stop=(c == 15),
                    )
            s1 = work1.tile([128, 4], f32, tag="s1")
            s1p = work1.tile([128, 4], f32, tag="s1p")
            for mb in range(4):
                nc.scalar.activation(
                    s1p[:, mb : mb + 1],
                    ps1[:, mb : mb + 1],
                    AF.Identity,
                    bias=b6_sb[:, mb : mb + 1],
                    scale=g6_sb[:, mb : mb + 1],
                )
            nc.vector.scalar_tensor_tensor(
                s1[:], s1p[:], 0.2, s1p[:], op0=ALU.mult, op1=ALU.max
            )
            ps2 = ps.tile([128, 2], f32, tag="ps")
            for mb in range(2):
                for c in range(4):
                    nc.tensor.matmul(
                        ps2[:, mb : mb + 1],
                        wl2_sb[:, c, mb * 128 : (mb + 1) * 128],
                        s1[:, c : c + 1],
                        start=(c == 0),
                        stop=(c == 3),
                    )
            s2 = work1.tile([128, 2], f32, tag="s2")
            s2p = work1.tile([128, 2], f32, tag="s2p")
            for mb in range(2):
                nc.scalar.activation(
                    s2p[:, mb : mb + 1],
                    ps2[:, mb : mb + 1],
                    AF.Identity,
                    bias=bi2_sb[:, mb : mb + 1],
                    scale=g7_sb[:, mb : mb + 1],
                )
            nc.vector.scalar_tensor_tensor(
                s2[:], s2p[:], 0.2, s2p[:], op0=ALU.mult, op1=ALU.max
            )
            ps3 = ps.tile([40, 1], f32, tag="ps")
            for c in range(2):
                nc.tensor.matmul(
                    ps3[:],
                    wl3_sb[0:128, c, :],
                    s2[:, c : c + 1],
                    start=(c == 0),
                    stop=(c == 1),
                )
            osb = work1.tile([40, 1], f32, tag="osb")
            nc.vector.tensor_tensor(osb[:], ps3[:], bl3_sb[:], ALU.add)
            nc.sync.dma_start(OUT[:], osb[:])

    nc.compile()
    return nc


def _get_module():
    if "nc" not in _CACHE:
        _CACHE["nc"] = _build_module()
    return _CACHE["nc"]


def _prep_weights(inp):
    """Host-side weight preprocessing -> dict of DRAM tensor arrays (fp32)."""
    f = np.float32
    out = {}
    ws = [
        (inp["w1"], inp["g1"], inp["b1"]),
        (inp["w2"], inp["g2"], inp["b2"]),
        (inp["w3"], inp["g3"], inp["b3"]),
        (inp["w4"], inp["g4"], inp["b4"]),
    ]
    for li, ((w, g, b), (ci, co)) in enumerate(zip(ws, DIMS)):
        w = np.asarray(w, f)
        nb = (co + 127) // 128
        cb = min(co, 128)
        out[f"wn{li}"] = np.ascontiguousarray(w[:, :ci].T)
        out[f"wd{li}"] = np.ascontiguousarray((w[:, ci:] - w[:, :ci]).T)
        out[f"gs{li}"] = np.ascontiguousarray(
            (np.asarray(g, f) * BN_SCALE).reshape(nb, cb).T
        )
        out[f"bb{li}"] = np.ascontiguousarray(np.asarray(b, f).reshape(nb, cb).T)
    w5 = np.asarray(inp["w5"], f)  # [1024, 512]
    w5t = w5.T  # [512, 1024]
    w5t_rs = np.zeros((128, 5, 1024), f)
    w5t_rs[0:64, 0, :] = w5t[0:64]
    w5t_rs[0:64, 1, :] = w5t[64:128]
    w5t_rs[:, 2, :] = w5t[128:256]
    w5t_rs[:, 3, :] = w5t[256:384]
    w5t_rs[:, 4, :] = w5t[384:512]
    out["w5t"] = w5t_rs
    out["g5s"] = np.ascontiguousarray(
        (np.asarray(inp["g5"], f) * BN_SCALE).reshape(8, 128).T
    )
    out["b5s"] = np.ascontiguousarray(np.asarray(inp["b5"], f).reshape(8, 128).T)
    wl1 = np.asarray(inp["wl1"], f).copy()  # [512, 2048]
    wl1[:, 1024:] *= f(1.0 / N)  # fold the mean-pool division
    out["wl1t"] = np.ascontiguousarray(wl1.T.reshape(16, 128, 512).transpose(1, 0, 2))
    out["g6s"] = np.ascontiguousarray(
        (np.asarray(inp["g6"], f) * BN_SCALE).reshape(4, 128).T
    )
    out["b6s"] = np.ascontiguousarray(np.asarray(inp["b6"], f).reshape(4, 128).T)
    wl2 = np.asarray(inp["wl2"], f)  # [256, 512]
    out["wl2t"] = np.ascontiguousarray(wl2.T.reshape(4, 128, 256).transpose(1, 0, 2))
    g7s = np.asarray(inp["g7"], f) * BN_SCALE
    out["g7s"] = np.ascontiguousarray(g7s.reshape(2, 128).T)
    bi2 = np.asarray(inp["bl2"], f) * g7s + np.asarray(inp["b7"], f)
    out["bi2"] = np.ascontiguousarray(bi2.reshape(2, 128).T)
    wl3 = np.asarray(inp["wl3"], f)  # [40, 256]
    out["wl3t"] = np.ascontiguousarray(wl3.T.reshape(2, 128, 40).transpose(1, 0, 2))
    out["bl3s"] = np.ascontiguousarray(np.asarray(inp["bl3"], f).reshape(40, 1))
    return out


def kernel(**inputs):
    from concourse.bass_utils import run_bass_kernel_spmd

    nc = _get_module()
    wmap = _prep_weights(inputs)
    x = np.asarray(inputs["x"], np.float32)  # [8, 3, 2048]
    in_maps = []
    for c in range(NCORES):
        m = dict(wmap)
        m["xq"] = np.ascontiguousarray(x[c])
        in_maps.append(m)
    res = run_bass_kernel_spmd(nc, in_maps, core_ids=list(range(NCORES)))
    out = np.stack([res.results[c]["outq"].reshape(40) for c in range(NCORES)])
    return out.astype(np.float32)


if __name__ == "__main__":
    nc = _get_module()
    print("module built OK")



# revision 12
# speedup vs baseline: 1.0733x; 1.0733x over previous
"""DGCNN kernel for Trainium2 (Bass/Tile), data-parallel over batch across 8 cores.

v2: fp16 matmuls, channel-grouped d=8 ap_gather with per-core point subsets,
PE-based ungroup (one-hot selector matmuls) that also absorbs the center term,
BN/ReLU fused in the PSUM->SBUF activation.

Per core, one point cloud of N=2048 points:
  EdgeConv decomposes as out[n] = relu(bn(max_k (Wn.p_idx) + Wd.p_n)),
  Wn = W[:, :C], Wd = W[:, C:] - W[:, :C].
  kNN keys[n,m] = p_n.p_m - 0.5|p_m|^2 (row term dropped: constant per row);
  exact top-24 per row via 3 rounds of DVE max8/max_index/match_replace.

Gather: Y stored channel-grouped "rep[16c+q, m, jj] = Y[8q+jj, m]" replicated
to all 8 core-groups; each GPSIMD core gathers d=8 words for its own 256-point
subset, so per-call read-command count drops 8x vs per-channel-column gathers.
Reduce(max over k) -> mtg (grouped); ungroup via 16-partition one-hot matmuls
into PSUM, center term accumulates into the same PSUM, ACT applies bn+relu.
"""

import sys

import numpy as np

sys.path.insert(0, "/opt/trn_rl_repo")

EPS = 1e-5
BN_SCALE = np.float32(1.0 / np.sqrt(1.0 + EPS))
N = 2048
K = 20
KR = 24
NCORES = 8
DIMS = [(3, 64), (64, 64), (64, 128), (128, 256)]  # (Cin, Cout)

_CACHE = {}


def _build_module():
    from concourse import bacc, mybir, tile

    dt = mybir.dt
    f32 = dt.float32
    f16 = dt.float16
    u16 = dt.uint16
    i16 = dt.int16
    AF = mybir.ActivationFunctionType
    ALU = mybir.AluOpType
    AX = mybir.AxisListType

    nc = bacc.Bacc("TRN2", target_bir_lowering=False, debug=False)

    # ---------------- DRAM I/O ----------------
    X = nc.dram_tensor("xq", [3, N], f16, kind="ExternalInput")
    conv_w = []
    for li, (ci, co) in enumerate(DIMS):
        nq = 32 if li == 3 else 16
        nb = (co + 127) // 128
        cb = min(co, 128)
        conv_w.append(
            dict(
                wnj=nc.dram_tensor(f"wnj{li}", [ci, 8, nq], f16, kind="ExternalInput"),
                wdT=nc.dram_tensor(f"wd{li}", [ci, co], f16, kind="ExternalInput"),
                gs=nc.dram_tensor(f"gs{li}", [cb, nb], f32, kind="ExternalInput"),
                bb=nc.dram_tensor(f"bb{li}", [cb, nb], f32, kind="ExternalInput"),
            )
        )
    EE = nc.dram_tensor("ee", [128, 8, 128], f16, kind="ExternalInput")
    EE4 = nc.dram_tensor("ee4", [128, 8, 256], f16, kind="ExternalInput")
    W5 = nc.dram_tensor("w5t", [128, 5, 1024], f16, kind="ExternalInput")
    G5 = nc.dram_tensor("g5s", [128, 8], f32, kind="ExternalInput")
    B5 = nc.dram_tensor("b5s", [128, 8], f32, kind="ExternalInput")
    WL1 = nc.dram_tensor("wl1t", [128, 16, 512], f16, kind="ExternalInput")
    G6 = nc.dram_tensor("g6s", [128, 4], f32, kind="ExternalInput")
    B6 = nc.dram_tensor("b6s", [128, 4], f32, kind="ExternalInput")
    WL2 = nc.dram_tensor("wl2t", [128, 4, 256], f16, kind="ExternalInput")
    G7 = nc.dram_tensor("g7s", [128, 2], f32, kind="ExternalInput")
    BI2 = nc.dram_tensor("bi2", [128, 2], f32, kind="ExternalInput")
    WL3 = nc.dram_tensor("wl3t", [128, 2, 40], f16, kind="ExternalInput")
    BL3 = nc.dram_tensor("bl3s", [40, 1], f32, kind="ExternalInput")
    OUT = nc.dram_tensor("outq", [40, 1], f32, kind="ExternalOutput")

    with tile.TileContext(nc) as tc:
        with (
            tc.tile_pool(name="const", bufs=1) as constp,
            tc.tile_pool(name="wts", bufs=1) as wts,
            tc.tile_pool(name="feat", bufs=1) as featp,
            tc.tile_pool(name="keysp", bufs=3) as keysp,
            tc.tile_pool(name="work", bufs=2) as work,
            tc.tile_pool(name="work1", bufs=1) as work1,
            tc.tile_pool(name="repp", bufs=1) as repp,
            tc.tile_pool(name="gat", bufs=2) as gatp,
            tc.tile_pool(name="pgp", bufs=6, space="PSUM") as pgp,
            tc.tile_pool(name="psB", bufs=2, space="PSUM") as psB,
        ):
            # ---------------- constants / weights to SBUF ----------------
            ones_h = constp.tile([1, 512], f16)
            nc.vector.memset(ones_h[:], 1.0)
            ones_col_h = constp.tile([128, 1], f16)
            nc.vector.memset(ones_col_h[:], 1.0)

            ee_sb = wts.tile([128, 8, 128], f16, tag="ee")
            nc.sync.dma_start(ee_sb[:], EE[:])
            ee4_sb = wts.tile([128, 8, 256], f16, tag="ee4")
            nc.sync.dma_start(ee4_sb[:], EE4[:])

            wnj_sb, wd_sb, gs_sb, bb_sb = [], [], [], []
            for li, (ci, co) in enumerate(DIMS):
                nq = 32 if li == 3 else 16
                nb = (co + 127) // 128
                cbp = min(co, 128)
                t_wnj = wts.tile([ci, 8, nq], f16, tag=f"wnj{li}")
                t_wd = wts.tile([ci, co], f16, tag=f"wd{li}")
                t_gs = wts.tile([cbp, nb], f32, tag=f"gs{li}")
                t_bb = wts.tile([cbp, nb], f32, tag=f"bb{li}")
                nc.sync.dma_start(t_wnj[:], conv_w[li]["wnj"][:])
                nc.sync.dma_start(t_wd[:], conv_w[li]["wdT"][:])
                nc.sync.dma_start(t_gs[:], conv_w[li]["gs"][:])
                nc.sync.dma_start(t_bb[:], conv_w[li]["bb"][:])
                wnj_sb.append(t_wnj)
                wd_sb.append(t_wd)
                gs_sb.append(t_gs)
                bb_sb.append(t_bb)

            w5_sb = wts.tile([128, 5, 1024], f16, tag="w5")
            nc.sync.dma_start(w5_sb[:], W5[:])
            g5_sb = wts.tile([128, 8], f32, tag="g5")
            b5_sb = wts.tile([128, 8], f32, tag="b5")
            nc.sync.dma_start(g5_sb[:], G5[:])
            nc.sync.dma_start(b5_sb[:], B5[:])
            wl1_sb = wts.tile([128, 16, 512], f16, tag="wl1")
            nc.sync.dma_start(wl1_sb[:], WL1[:])
            g6_sb = wts.tile([128, 4], f32, tag="g6")
            b6_sb = wts.tile([128, 4], f32, tag="b6")
            nc.sync.dma_start(g6_sb[:], G6[:])
            nc.sync.dma_start(b6_sb[:], B6[:])
            wl2_sb = wts.tile([128, 4, 256], f16, tag="wl2")
            nc.sync.dma_start(wl2_sb[:], WL2[:])
            g7_sb = wts.tile([128, 2], f32, tag="g7")
            bi2_sb = wts.tile([128, 2], f32, tag="bi2")
            nc.sync.dma_start(g7_sb[:], G7[:])
            nc.sync.dma_start(bi2_sb[:], BI2[:])
            wl3_sb = wts.tile([128, 2, 40], f16, tag="wl3")
            nc.sync.dma_start(wl3_sb[:], WL3[:])
            bl3_sb = wts.tile([40, 1], f32, tag="bl3")
            nc.sync.dma_start(bl3_sb[:], BL3[:])

            pt0 = featp.tile([3, N], f16, tag="pt0")
            nc.sync.dma_start(pt0[:], X[:])

            f1 = featp.tile([64, N], f16, tag="f1")
            f2 = featp.tile([64, N], f16, tag="f2")
            f3 = featp.tile([128, N], f16, tag="f3")
            f4a = featp.tile([128, N], f16, tag="f4a")
            f4b = featp.tile([128, N], f16, tag="f4b")

            rep = repp.tile([128, N, 8], f16, tag="rep")

            def edge_layer(li, PT, out_aps):
                """PT: AP [Cin, N] fp16. out_aps: list of APs [cb, N] per block."""
                ci, co = DIMS[li]
                nq = 32 if li == 3 else 16  # rep channel-groups per replica
                nrep = 4 if li == 3 else 8  # replicas
                ntl = 16 // nrep  # tiles (128-pt) per replica subset
                ee = ee4_sb if li == 3 else ee_sb

                # ---- rep build: rep[g*nq+q, m, jj] = Y[8q+jj, m] ----
                # (for li==3, q in 0..31 spans both 128-blocks; groups are
                #  32-partition replicas. Y values never materialized flat.)
                for jj in range(8):
                    for j4 in range(4):
                        sl = slice(j4 * 512, (j4 + 1) * 512)
                        pr = psB.tile([nq, 512], f32, tag="psb")
                        nc.tensor.matmul(
                            pr[:],
                            wnj_sb[li][:, jj, :],
                            PT[:, sl],
                            start=True,
                            stop=True,
                        )
                        nc.scalar.activation(
                            rep[0:nq, sl, jj], pr[:], AF.Copy
                        )
                # replicate rep[0:nq] to the other replicas
                for g in range(1, nrep):
                    nc.sync.dma_start(
                        rep[g * nq : (g + 1) * nq, :, :], rep[0:nq, :, :]
                    )

                # ---- squared norms: nsq = -0.5*|p_m|^2 ----
                p2 = work.tile([ci, N], f16, tag="p2")
                nc.scalar.activation(p2[:], PT, AF.Square)
                nsq = work1.tile([1, N], f16, tag="nsq")
                for j4 in range(4):
                    sl = slice(j4 * 512, (j4 + 1) * 512)
                    pq = psB.tile([1, 512], f32, tag="psb")
                    nc.tensor.matmul(
                        pq[:], ones_col_h[0:ci, :], p2[:, sl], start=True, stop=True
                    )
                    nc.scalar.activation(nsq[:, sl], pq[:], AF.Copy, scale=-0.5)

                # ---- distance keys + exact top-24 per row ----
                dec = work1.tile([128, 16, KR], u16, tag="dec")
                for t in range(16):
                    tl = slice(t * 128, (t + 1) * 128)
                    keys = keysp.tile([128, N], f32, tag="keys")
                    for j4 in range(4):
                        sl = slice(j4 * 512, (j4 + 1) * 512)
                        pg = pgp.tile([128, 512], f32, tag="pg")
                        nc.tensor.matmul(
                            pg[:], PT[:, tl], PT[:, sl], start=True, stop=False
                        )
                        nc.tensor.matmul(
                            pg[:],
                            ones_h[:, 0:128],
                            nsq[:, sl],
                            start=False,
                            stop=True,
                        )
                        nc.scalar.activation(keys[:, sl], pg[:], AF.Copy)
                    for r in range(3):
                        v8 = work.tile([128, 8], f32, tag="v8")
                        nc.vector.max(v8[:], keys[:])
                        nc.vector.max_index(
                            dec[:, t, r * 8 : (r + 1) * 8], v8[:], keys[:]
                        )
                        if r < 2:
                            nc.vector.match_replace(keys[:], v8[:], keys[:], -3.0e38)

                # ---- fold indices: w0[p, t, g, k] = dec[16g+p, t, k] ----
                w0 = work1.tile([16, 16, 8, K], i16, tag="w0")
                for g in range(8):
                    nc.sync.dma_start(
                        w0[:, :, g, :], dec[16 * g : 16 * (g + 1), :, 0:K].bitcast(i16)
                    )
                # distribute per-replica subsets, replicated to its cores
                # widx[16c+p, (t_loc, g, k)] = w0[p, subset_tiles(c), g, k]
                widx = work1.tile([128, ntl, 8, K], i16, tag="widx")
                for c in range(8):
                    t0 = (c // (8 // nrep)) * ntl
                    nc.sync.dma_start(
                        widx[16 * c : 16 * (c + 1), :, :, :],
                        w0[:, t0 : t0 + ntl, :, :],
                    )

                # ---- gathers (per half-tile-group) + reduce into mtg ----
                # per-core call s covers f-slice [s*80, s*80+80) = (t_loc, g-half)
                # JC = 1280 idxs/core/call -> 4 calls (li<3) / 8 calls (li==3)
                ncalls = 2 * ntl
                mtg = work1.tile([128, ntl * 8, 128], f16, tag="mtg")
                for s in range(ncalls):
                    g = gatp.tile([128, 1280, 8], f16, tag="g")
                    nc.gpsimd.ap_gather(
                        g[:],
                        rep[:],
                        widx[:].rearrange("p a b k -> p (a b k)")[
                            :, s * 80 : (s + 1) * 80
                        ],
                        channels=128,
                        num_elems=N,
                        d=8,
                        num_idxs=1280,
                    )
                    # g free = j*8+jj, j = (gh 4, k 20, p~ 16); reduce over k
                    gv = g[:].rearrange("p (gh k x) j -> p gh (x j) k", gh=4, k=K)
                    nc.vector.tensor_reduce(
                        mtg[:, s * 4 : (s + 1) * 4, :], gv, axis=AX.X, op=ALU.max
                    )

                # ---- ungroup + center term + bn/relu, per 128-pt tile ----
                # mtg[R*nq+q, tg, p~*8+jj] = max_k Y[8q+jj, n],
                #   n = subset(R) + tg*16... (flattened: per replica R the
                #   covered points are R*ntl*128 ... in (t_loc, g, p~) order)
                nblk = (co + 127) // 128
                for R in range(nrep):
                    # PE matmul operands must sit at base partition 0; DMA-shift
                    # replicas' mtg rows down, then planarize jj on DVE so the
                    # matmul rhs is contiguous.
                    base = R * nq
                    if base == 0:
                        m0i = mtg[0:nq, :, :]
                    else:
                        m0it = work1.tile([nq, ntl * 8, 128], f16, tag="mtg0")
                        nc.sync.dma_start(m0it[:], mtg[base : base + nq, :, :])
                        m0i = m0it[:]
                    m0p = work1.tile([nq, 8, ntl * 128], f16, tag="mtgp")
                    nc.scalar.activation(
                        m0p[:], m0i.rearrange("q a (x j) -> q j (a x)", j=8), AF.Copy
                    )
                    eeR = ee[0:nq, :, :]
                    for t_loc in range(ntl):
                        n0 = (R * ntl + t_loc) * 128
                        for blk in range(nblk):
                            cb = min(co - blk * 128, 128)
                            csl = slice(blk * 128, blk * 128 + cb)
                            pu = psB.tile([cb, 128], f32, tag="psb")
                            for jj in range(8):
                                nc.tensor.matmul(
                                    pu[:],
                                    eeR[:, jj, csl],
                                    m0p[:, jj, t_loc * 128 : (t_loc + 1) * 128],
                                    start=(jj == 0),
                                    stop=False,
                                )
                            nc.tensor.matmul(
                                pu[:],
                                wd_sb[li][:, csl],
                                PT[:, n0 : n0 + 128],
                                start=False,
                                stop=True,
                            )
                            nc.scalar.activation(
                                out_aps[blk][:, n0 : n0 + 128],
                                pu[:],
                                AF.Relu,
                                bias=bb_sb[li][0:cb, blk : blk + 1],
                                scale=gs_sb[li][0:cb, blk : blk + 1],
                            )

            edge_layer(0, pt0[:], [f1[:]])
            edge_layer(1, f1[:], [f2[:]])
            edge_layer(2, f2[:], [f3[:]])
            edge_layer(3, f3[:], [f4a[:], f4b[:]])

            # ---------------- conv5 (1024) + global max/mean pool ----------------
            pooled = work1.tile([128, 16], f32, tag="pooled")
            rhs_chunks = [f1[:], f2[:], f3[:], f4a[:], f4b[:]]
            chunk_rows = [64, 64, 128, 128, 128]
            for blk in range(8):
                bsl = slice(blk * 128, (blk + 1) * 128)
                hb = gatp.tile([128, N], f32, tag="g")
                for j4 in range(4):
                    sl = slice(j4 * 512, (j4 + 1) * 512)
                    ph = psB.tile([128, 512], f32, tag="psb")
                    for c in range(5):
                        nc.tensor.matmul(
                            ph[:],
                            w5_sb[0 : chunk_rows[c], c, bsl],
                            rhs_chunks[c][:, sl],
                            start=(c == 0),
                            stop=(c == 4),
                        )
                    nc.scalar.activation(
                        hb[:, sl],
                        ph[:],
                        AF.Relu,
                        bias=b5_sb[:, blk : blk + 1],
                        scale=g5_sb[:, blk : blk + 1],
                    )
                nc.vector.tensor_reduce(
                    pooled[:, blk : blk + 1], hb[:], axis=AX.X, op=ALU.max
                )
                nc.vector.tensor_reduce(
                    pooled[:, 8 + blk : 9 + blk], hb[:], axis=AX.X, op=ALU.add
                )

            # ---------------- MLP head ----------------
            pooled_h = work1.tile([128, 16], f16, tag="pooledh")
            nc.scalar.activation(pooled_h[:], pooled[:], AF.Copy)
            ps1 = psB.tile([128, 4], f32, tag="psb")
            for mb in range(4):
                for c in range(16):
                    nc.tensor.matmul(
                        ps1[:, mb : mb + 1],
                        wl1_sb[:, c, mb * 128 : (mb + 1) * 128],
                        pooled_h[:, c : c + 1],
                        start=(c == 0),
                        stop=(c == 15),
                    )
            s1p = work1.tile([128, 4], f32, tag="s1p")
            s1 = work1.tile([128, 4], f16, tag="s1")
            for mb in range(4):
                nc.scalar.activation(
                    s1p[:, mb : mb + 1],
                    ps1[:, mb : mb + 1],
                    AF.Identity,
                    bias=b6_sb[:, mb : mb + 1],
                    scale=g6_sb[:, mb : mb + 1],
                )
            nc.vector.scalar_tensor_tensor(
                s1[:], s1p[:], 0.2, s1p[:], op0=ALU.mult, op1=ALU.max
            )
            ps2 = psB.tile([128, 2], f32, tag="psb")
            for mb in range(2):
                for c in range(4):
                    nc.tensor.matmul(
                        ps2[:, mb : mb + 1],
                        wl2_sb[:, c, mb * 128 : (mb + 1) * 128],
                        s1[:, c : c + 1],
                        start=(c == 0),
                        stop=(c == 3),
                    )
            s2p = work1.tile([128, 2], f32, tag="s2p")
            s2 = work1.tile([128, 2], f16, tag="s2")
            for mb in range(2):
                nc.scalar.activation(
                    s2p[:, mb : mb + 1],
                    ps2[:, mb : mb + 1],
                    AF.Identity,
                    bias=bi2_sb[:, mb : mb + 1],
                    scale=g7_sb[:, mb : mb + 1],
                )
            nc.vector.scalar_tensor_tensor(
                s2[:], s2p[:], 0.2, s2p[:], op0=ALU.mult, op1=ALU.max
            )
            ps3 = psB.tile([40, 1], f32, tag="psb")
            for c in range(2):
                nc.tensor.matmul(
                    ps3[:],
                    wl3_sb[0:128, c, :],
                    s2[:, c : c + 1],
                    start=(c == 0),
                    stop=(c == 1),
                )
            osb = work1.tile([40, 1], f32, tag="osb")
            nc.vector.tensor_tensor(osb[:], ps3[:], bl3_sb[:], ALU.add)
            nc.sync.dma_start(OUT[:], osb[:])

    nc.compile()
    return nc
